# revision 4
# baseline (speedup 1.0000x reference)
"""Trainium2 Bass kernel v2 for nn_Block_56126632624726 (dense transformer block).

Reference computation (fp32, B=4, L=2048, D=1024, H=8 heads, hd=128):
    h = LayerNorm(x) * gamma + beta
    [q,k,v,lin,pre] = h @ w_qkv.T            (5*D outputs)
    attn = causal p-softmax attention (p=2)
    branch = [lin * gelu(pre), attn]
    out = x + branch @ w_out.T

Sharding: 8 cores = 4 batches (data parallel) x 2 tensor-parallel halves.
Core j in {0,1} of a batch owns heads 4j..4j+3 plus lin/pre cols
512j..512j+512 and the matching w_out input columns; host sums the two
partial outputs per batch and adds the residual (no device collectives).

v2 design (vs the fp32r v1):
  - fp16 operands everywhere on the PE (1 cyc/row vs ~1.6 for fp32r),
    fp32 PSUM accumulation. gamma folded into w_qkv host-side; beta enters
    as per-partition biases during PSUM evacuation.
  - All weights SBUF-resident (loaded once, fp16): no per-quarter DMA.
  - LN: token sums via ones-matmuls; rsqrt via Act Sqrt + DVE
    reciprocal_approx_fast; per-token scale/shift broadcast across
    partitions with a PE ones-broadcast (no DRAM bounce).
  - Attention per (quarter J, head h) with f' = exp(s*scale - 8*ln2)
    (fp16-safe rescale; cancels in the p=2 normalization), diagonal tiles
    trimmed to the causal query range, r accumulated for all 4 heads of a
    quarter in one [4,512] PSUM tile via an indicator lhsT, o evacuated
    UNNORMALIZED (scaled by 1/4) and normalized at the end from
    cr = rsqrt(r) with a PE broadcast of 4*cr.
  - Out-projection from branchT = [gT; attnT] with fp16 w_out slice,
    fp16 partial output (host upcasts and adds the residual).
"""

import numpy as np

B = 4
L = 2048
D = 1024
P = 128
KC = D // P  # 8 dim chunks
NQ = 4
TQ = L // NQ  # 512
NT = L // P  # 16 token tiles
HL = 4  # heads per core
HD = 128
SCALE = float(HD) ** -0.5
EXPB = -4.0 * float(np.log(2.0))  # exp bias: f' = f * 2^-4 (fp16 range safety)
EPS = 1e-5

_CACHED = {}


def _install_tile_drain_patch(tile, mybir):
    """walrus limits sem waits per SP CTRL instruction to 1; split the
    TileContext final drain's waits across sequential drain instructions."""
    from concourse.vector_clock import ScopedClock

    if getattr(tile.TileContext, "_drain_patched", False):
        return

    def _patched(self, tick_clock, wait_clock):
        drain_inst = self.nc.sync.drain()
        wait_clock.add_sem_waits(
            drain_inst.ins, ScopedClock({None: tick_clock.global_clock})
        )
        si = drain_inst.ins.sync_info
        waits = list(si.on_wait or []) if si else []
        if len(waits) > 1:
            si.on_wait = waits[:1]
            for w in waits[1:]:
                d2 = self.nc.sync.drain()
                d2.ins.sync_info = mybir.SyncInfo(on_wait=[w], on_update=[])
        self.nc.all_engine_barrier()
        popped = self.nc._tile_sem_poison_stack.pop()
        assert popped is self._sem_poison
        self.nc.clear_and_free_semaphores(list(self.sems.allocated().values()))
        self.nc.all_engine_barrier()

    tile.TileContext._drain_and_barrier = _patched
    tile.TileContext._drain_patched = True


def _split_multi_waits(nc, mybir):
    """This walrus build supports at most ONE sync-wait per instruction.
    Move extra waits onto same-engine nops inserted before the instruction."""
    eng_builder = {
        mybir.EngineType.PE: nc.tensor,
        mybir.EngineType.DVE: nc.vector,
        mybir.EngineType.Activation: nc.scalar,
        mybir.EngineType.SP: nc.sync,
        mybir.EngineType.Pool: nc.gpsimd,
    }

    def make_nop(engine):
        bi = eng_builder[engine].nop(nofuse=True)
        inst = bi.ins
        nc.cur_bb.bb.instructions.remove(inst)
        return inst

    for f in nc.m.functions:
        for bb in f.blocks:
            insts = bb.instructions
            rebuilt = []
            changed = False
            for inst in list(insts):
                si = inst.sync_info
                waits = list(si.on_wait or []) if si else []
                if len(waits) > 1:
                    changed = True
                    for w in waits[:-1]:
                        nop = make_nop(inst.engine)
                        nop.sync_info = mybir.SyncInfo(on_wait=[w], on_update=[])
                        rebuilt.append(nop)
                    si.on_wait = waits[-1:]
                rebuilt.append(inst)
            if changed:
                insts.clear()
                insts.extend(rebuilt)


def _build_nc(beta_zero=False):
    import concourse.bass as bass
    import concourse.tile as tile
    from concourse import mybir

    _install_tile_drain_patch(tile, mybir)

    f32 = mybir.dt.float32
    f16 = mybir.dt.float16
    bf16 = mybir.dt.bfloat16
    AF = mybir.ActivationFunctionType
    OP = mybir.AluOpType

    nc = bass.Bass()

    xT = nc.declare_dram_parameter("xT16", [D, L], f16, isOutput=False)
    wq = nc.declare_dram_parameter("wq16", [D, TQ], f16, isOutput=False)
    wk = nc.declare_dram_parameter("wk16", [D, TQ], f16, isOutput=False)
    wv = nc.declare_dram_parameter("wv16", [D, TQ], f16, isOutput=False)
    wl = nc.declare_dram_parameter("wl16", [D, TQ], f16, isOutput=False)
    wp = nc.declare_dram_parameter("wp16", [D, TQ], f16, isOutput=False)
    wo = nc.declare_dram_parameter("wo16", [D, D], f16, isOutput=False)
    bqk = nc.declare_dram_parameter("bqk", [P, 8], f32, isOutput=False)
    blp = nc.declare_dram_parameter("blp", [P, 8], f32, isOutput=False)
    bv = nc.declare_dram_parameter("bv16", [1, TQ], f16, isOutput=False)
    maskT = nc.declare_dram_parameter("mask16", [P, 896], f16, isOutput=False)
    out = nc.declare_dram_parameter("out16", [L, D], f16, isOutput=True)

    x_r = xT.rearrange("(o p) t -> p o t", p=P)  # [128, 8, 2048]
    wq_r = wq.rearrange("(o p) f -> p o f", p=P)  # [128, 8, 512]
    wk_r = wk.rearrange("(o p) f -> p o f", p=P)
    wv_r = wv.rearrange("(o p) f -> p o f", p=P)
    wl_r = wl.rearrange("(o p) f -> p o f", p=P)
    wp_r = wp.rearrange("(o p) f -> p o f", p=P)
    wo_r = wo.rearrange("(o p) f -> p o f", p=P)  # [128, 8, 1024]

    with tile.TileContext(nc) as tc:
        with tc.tile_pool(name="persist", bufs=1) as ps_pool:
            # ---- persistent SBUF residents ----
            xh = ps_pool.tile([P, KC, L], f16)  # x, normalized in place
            for _q in range(2):
                _qs = slice(TQ * _q, TQ * _q + TQ)
                nc.sync.dma_start(out=xh[:, 0:4, _qs], in_=x_r[:, 0:4, _qs])
                nc.sync.dma_start(out=xh[:, 4:8, _qs], in_=x_r[:, 4:8, _qs])
            wq_s = ps_pool.tile([P, KC, TQ], f16)
            nc.sync.dma_start(out=wq_s[:], in_=wq_r[:])
            for _q in range(2, NQ):
                _qs = slice(TQ * _q, TQ * _q + TQ)
                nc.sync.dma_start(out=xh[:, 0:4, _qs], in_=x_r[:, 0:4, _qs])
                nc.sync.dma_start(out=xh[:, 4:8, _qs], in_=x_r[:, 4:8, _qs])
            wk_s = ps_pool.tile([P, KC, TQ], f16)
            nc.sync.dma_start(out=wk_s[:], in_=wk_r[:])
            wv_s = ps_pool.tile([P, KC, TQ], f16)
            nc.sync.dma_start(out=wv_s[:], in_=wv_r[:])
            wl_s = ps_pool.tile([P, KC, TQ], f16)
            nc.sync.dma_start(out=wl_s[:], in_=wl_r[:])
            wp_s = ps_pool.tile([P, KC, TQ], f16)
            nc.sync.dma_start(out=wp_s[:], in_=wp_r[:])
            wo_s = ps_pool.tile([P, KC, D], f16)
            nc.sync.dma_start(out=wo_s[:], in_=wo_r[:])
            masks = ps_pool.tile([P, 896], f16)
            nc.sync.dma_start(out=masks[:], in_=maskT[:])
            bqk_s = ps_pool.tile([P, 8], f32)
            nc.sync.dma_start(out=bqk_s[:], in_=bqk[:])
            blp_s = ps_pool.tile([P, 8], f32)
            nc.sync.dma_start(out=blp_s[:], in_=blp[:])
            bv_s = ps_pool.tile([1, TQ], f16)
            nc.sync.dma_start(out=bv_s[:], in_=bv[:])

            QT = ps_pool.tile([P, HL, L], f16)
            KT = ps_pool.tile([P, HL, L], f16)
            V = ps_pool.tile([P, NT, TQ], f16)  # token-major
            gT = ps_pool.tile([P, HL, L], f16)
            attnT = ps_pool.tile([P, HL, L], f16)  # o'/4, normalized in place

            onesc = ps_pool.tile([P, 1], f16)
            nc.vector.memset(onesc, 1.0)
            onesb = ps_pool.tile([P, 1], bf16)
            nc.vector.memset(onesb, 1.0)
            onesr = ps_pool.tile([1, P], f16)
            nc.vector.memset(onesr, 1.0)
            epst = ps_pool.tile([1, 1], f32)
            nc.vector.memset(epst, EPS)
            nl4 = ps_pool.tile([1, 1], f32)
            nc.vector.memset(nl4, -float(np.log(4.0)))
            expb = ps_pool.tile([P, 1], f32)
            nc.vector.memset(expb, EXPB)

            # =========== Phase 1: LN + projections ===========
            with (
                tc.tile_pool(name="sq", bufs=2) as sqp,
                tc.tile_pool(name="rows", bufs=2) as rows_p,
                tc.tile_pool(name="bc", bufs=3) as bcp,
                tc.tile_pool(name="gel", bufs=2) as gelp,
                tc.tile_pool(name="lin", bufs=2) as linp,
                tc.tile_pool(name="rps", bufs=2, space="PSUM") as rps,
                tc.tile_pool(name="bps", bufs=1, space="PSUM") as bps,
                tc.tile_pool(name="pps", bufs=2, space="PSUM") as pps,
            ):

                def stats(q):
                    qsl = slice(TQ * q, TQ * q + TQ)
                    s1 = rps.tile([1, TQ], f32, tag="s1", name=f"s1_{q}")
                    s2 = rps.tile([1, TQ], f32, tag="s2", name=f"s2_{q}")
                    for k in range(KC):
                        nc.tensor.matmul(
                            s1, lhsT=onesc[:], rhs=xh[:, k, qsl],
                            start=(k == 0), stop=(k == KC - 1),
                        )
                    for k in range(KC):
                        x2 = sqp.tile([P, TQ], f16, tag="x2")
                        nc.vector.tensor_mul(
                            out=x2[:], in0=xh[:, k, qsl], in1=xh[:, k, qsl]
                        )
                        nc.tensor.matmul(
                            s2, lhsT=onesc[:], rhs=x2[:],
                            start=(k == 0), stop=(k == KC - 1),
                        )
                    return s1, s2

                def rows_math(q, s1, s2):
                    # mu/var/inv rows + fp16 cast (act/dve only, no PE)
                    mu = rows_p.tile([1, TQ], f32, tag="mu")
                    nc.scalar.mul(out=mu[:], in_=s1, mul=1.0 / D)
                    m2 = rows_p.tile([1, TQ], f32, tag="m2")
                    nc.scalar.mul(out=m2[:], in_=s2, mul=1.0 / D)
                    var = rows_p.tile([1, TQ], f32, tag="var")
                    nc.vector.tensor_mul(out=var[:], in0=mu[:], in1=mu[:])
                    nc.vector.tensor_tensor(
                        out=var[:], in0=m2[:], in1=var[:], op=OP.subtract
                    )
                    # inv = rsqrt(var+eps) via ln->exp (both in one act table)
                    lnv = rows_p.tile([1, TQ], f32, tag="lnv")
                    nc.scalar.activation(
                        out=lnv[:], in_=var[:], func=AF.Ln, bias=epst[:]
                    )
                    inv16 = rows_p.tile([1, TQ], f16, tag="inv16", name=f"iv{q}")
                    nc.scalar.activation(
                        out=inv16[:], in_=lnv[:], func=AF.Exp, scale=-0.5
                    )
                    nmu = rows_p.tile([1, TQ], f32, tag="nmu")
                    nc.scalar.mul(out=nmu[:], in_=s1, mul=-1.0 / D)
                    ninv16 = rows_p.tile([1, TQ], f16, tag="ninv16", name=f"nv{q}")
                    nc.vector.tensor_mul(out=ninv16[:], in0=nmu[:], in1=inv16[:])
                    return inv16, ninv16

                def bcast(q, inv16, ninv16):
                    inb_ps = bps.tile([P, TQ], f32, tag="inb", name=f"inb{q}")
                    nc.tensor.matmul(
                        inb_ps, lhsT=onesr[:], rhs=inv16[:], start=True, stop=True
                    )
                    nnb_ps = bps.tile([P, TQ], f32, tag="nnb", name=f"nnb{q}")
                    nc.tensor.matmul(
                        nnb_ps, lhsT=onesr[:], rhs=ninv16[:], start=True, stop=True
                    )
                    invb = bcp.tile([P, TQ], f16, tag="invb", name=f"ib{q}")
                    nc.scalar.copy(out=invb[:], in_=inb_ps)
                    ninvb = bcp.tile([P, TQ], f16, tag="ninvb", name=f"nb{q}")
                    nc.vector.tensor_copy(out=ninvb[:], in_=nnb_ps)
                    return invb, ninvb

                def normalize(q, invb, ninvb):
                    qsl = slice(TQ * q, TQ * q + TQ)
                    for k in range(KC):
                        nc.vector.tensor_mul(
                            out=xh[:, k, qsl], in0=xh[:, k, qsl], in1=invb[:]
                        )
                        nc.vector.tensor_add(
                            out=xh[:, k, qsl], in0=xh[:, k, qsl], in1=ninvb[:]
                        )

                def proj(q):
                    qsl = slice(TQ * q, TQ * q + TQ)
                    # q/k heads -> QT/KT (dim-major), Act Identity evac w/ bias
                    for dst, wsb, bcol0 in ((QT, wq_s, 0), (KT, wk_s, 4)):
                        for m in range(HL):
                            ps = pps.tile([P, TQ], f32, tag="mm")
                            for k in range(KC):
                                nc.tensor.matmul(
                                    ps, lhsT=wsb[:, k, P * m : P * m + P],
                                    rhs=xh[:, k, qsl],
                                    start=(k == 0), stop=(k == KC - 1),
                                )
                            nc.scalar.activation(
                                out=dst[:, m, qsl], in_=ps, func=AF.Identity,
                                bias=bqk_s[:, bcol0 + m : bcol0 + m + 1],
                            )
                    # gT = (lin+b) * gelu(pre+b)
                    for m in range(HL):
                        psp = pps.tile([P, TQ], f32, tag="mm")
                        for k in range(KC):
                            nc.tensor.matmul(
                                psp, lhsT=wp_s[:, k, P * m : P * m + P],
                                rhs=xh[:, k, qsl],
                                start=(k == 0), stop=(k == KC - 1),
                            )
                        gel = gelp.tile([P, TQ], f16, tag="gel")
                        nc.scalar.activation(
                            out=gel[:], in_=psp, func=AF.Gelu,
                            bias=blp_s[:, 4 + m : 4 + m + 1],
                        )
                        psl = pps.tile([P, TQ], f32, tag="mm")
                        for k in range(KC):
                            nc.tensor.matmul(
                                psl, lhsT=wl_s[:, k, P * m : P * m + P],
                                rhs=xh[:, k, qsl],
                                start=(k == 0), stop=(k == KC - 1),
                            )
                        lnb = linp.tile([P, TQ], f16, tag="lnb")
                        nc.vector.tensor_scalar(
                            out=lnb[:], in0=psl,
                            scalar1=blp_s[:, m : m + 1], scalar2=None,
                            op0=OP.add,
                        )
                        nc.vector.tensor_mul(
                            out=gT[:, m, qsl], in0=lnb[:], in1=gel[:]
                        )
                    # V (token-major) via swapped matmul, bias row via ones-matmul
                    for i in range(NQ):
                        vps = pps.tile([P, TQ], f32, tag="mm")
                        if not beta_zero:
                            nc.tensor.matmul(
                                vps, lhsT=onesr[:], rhs=bv_s[:],
                                start=True, stop=False,
                            )
                        tsl = slice(TQ * q + P * i, TQ * q + P * i + P)
                        for k in range(KC):
                            nc.tensor.matmul(
                                vps, lhsT=xh[:, k, tsl], rhs=wv_s[:, k, :],
                                start=(beta_zero and k == 0), stop=(k == KC - 1),
                            )
                        nc.vector.tensor_copy(out=V[:, NQ * q + i, :], in_=vps)

                # LN fully precomputed up front: stats matmuls hide the
                # act/dve row chains, normalizes hide behind later stats/proj.
                st0 = stats(0)
                st1 = stats(1)
                rw0 = rows_math(0, *st0)
                rw1 = rows_math(1, *st1)
                st2 = stats(2)
                st3 = stats(3)
                bc0 = bcast(0, *rw0)
                bc1 = bcast(1, *rw1)
                normalize(0, *bc0)
                normalize(1, *bc1)
                rw2 = rows_math(2, *st2)
                rw3 = rows_math(3, *st3)
                proj(0)
                bc2 = bcast(2, *rw2)
                bc3 = bcast(3, *rw3)
                normalize(2, *bc2)
                proj(1)
                normalize(3, *bc3)
                proj(2)
                proj(3)

            # =========== Phase 2+3: attention fused with out-projection ===========
            # attnT holds o'/4, normalized per quarter with cr = rsqrt(r')
            # (ln->exp on the act engine; exp/ln share a table so there is
            # no act-table thrash). The missing 4x is folded into w_out
            # host-side. cr rows are broadcast across partitions with a
            # DRAM-bounce DMA. Off-diagonal score tiles are processed in
            # pairs (one exp / one square per 1024 columns) to cut act-engine
            # overhead. Out-projection PSUM groups for quarter J-1 are
            # interleaved between head blocks of quarter J to fill PE gaps
            # while the act engine paces the exp chain; their PSUM->SBUF
            # evacuation runs on the otherwise-idle gpsimd engine.
            import concourse.bass as _b

            QS = (0, P, 2 * P, 3 * P)  # diagonal-tile query range starts
            with (
                tc.tile_pool(name="fp", bufs=4) as fp,
                tc.tile_pool(name="ep", bufs=3) as ep,
                tc.tile_pool(name="rbp", bufs=2) as rbp,
                tc.tile_pool(name="crp", bufs=2) as crp,
                tc.tile_pool(name="cbt", bufs=3) as cbtp,
                tc.tile_pool(name="obuf", bufs=3) as obuf,
                tc.tile_pool(name="rowd", bufs=3, space="DRAM") as rowd,
                tc.tile_pool(name="stps", bufs=3, space="PSUM") as stps,
                tc.tile_pool(name="ops", bufs=2, space="PSUM") as ops,
                tc.tile_pool(name="arps", bufs=1, space="PSUM") as arps,
                tc.tile_pool(name="p3ps", bufs=2, space="PSUM") as p3ps,
            ):
                def p3_group(i, n):
                    ps3 = p3ps.tile([P, TQ], f32, tag="o3")
                    for c in range(KC):
                        src3 = gT if c < HL else attnT
                        nc.tensor.matmul(
                            ps3,
                            lhsT=src3[:, c % HL, P * i : P * i + P],
                            rhs=wo_s[:, c, TQ * n : TQ * n + TQ],
                            start=(c == 0), stop=(c == KC - 1),
                        )
                    ot = obuf.tile([P, TQ], f16, tag="ot")
                    nc.vector.tensor_copy(out=ot[:], in_=ps3)
                    nc.sync.dma_start(
                        out=out[P * i : P * i + P, TQ * n : TQ * n + TQ],
                        in_=ot[:],
                    )

                p3_queue = []
                for J in range(NQ):
                    jsl = slice(TQ * J, TQ * J + TQ)
                    nt = 4 * J + 4
                    for h in range(HL):
                        o_ps = ops.tile([P, TQ], f32, tag="o")
                        r_ps = arps.tile([1, TQ], f32, tag="r")

                        def scores(t):
                            d = t - 4 * J
                            qs, ext = (QS[d], TQ - QS[d]) if d >= 0 else (0, TQ)
                            st = stps.tile([P, TQ], f32, tag="st")
                            nc.tensor.matmul(
                                st[:, qs : qs + ext],
                                lhsT=KT[:, h, P * t : P * t + P],
                                rhs=QT[:, h, TQ * J + qs : TQ * J + qs + ext],
                                start=True, stop=True,
                            )
                            f = fp.tile([P, TQ], f16, tag="f")
                            nc.scalar.activation(
                                out=f[:, :ext], in_=st[:, qs : qs + ext],
                                func=AF.Exp, scale=SCALE, bias=expb[:],
                            )
                            if d >= 0:
                                off = 384 - (P * d - qs)
                                nc.vector.tensor_mul(
                                    out=f[:, :ext], in0=f[:, :ext],
                                    in1=masks[:, off : off + ext],
                                )
                            e = ep.tile([P, TQ], bf16, tag="e")
                            nc.vector.tensor_mul(
                                out=e[:, :ext], in0=f[:, :ext], in1=f[:, :ext]
                            )
                            return (t, f, e, qs, ext)

                        def accum(unit):
                            t, f, e, qs, ext = unit
                            nc.tensor.matmul(
                                r_ps[:, qs : qs + ext],
                                lhsT=onesb[:],
                                rhs=e[:, :ext],
                                start=(t == 0), stop=(t == nt - 1),
                            )
                            nc.tensor.matmul(
                                o_ps[:, qs : qs + ext],
                                lhsT=V[:, t, P * h : P * h + P],
                                rhs=f[:, :ext],
                                start=(t == 0), stop=(t == nt - 1),
                            )

                        # software pipeline, depth 1: scores(t+1) then accum(t)
                        pend = scores(0)
                        for t in range(1, nt):
                            cur = scores(t)
                            accum(pend)
                            pend = cur
                        accum(pend)
                        # cr/4 = exp(-0.5*ln r - ln 4) straight off PSUM r,
                        # broadcast via DRAM bounce, then one TT evacuates
                        # o_ps normalized: attnT = o' * cr/4 (4x is in w_out)
                        lnr = rbp.tile([1, TQ], f32, tag="ln")
                        nc.scalar.activation(out=lnr[:], in_=r_ps, func=AF.Ln)
                        crh = crp.tile([1, TQ], f16, tag="cr")
                        nc.scalar.activation(
                            out=crh[:], in_=lnr[:], func=AF.Exp, scale=-0.5,
                            bias=nl4[:],
                        )
                        rd = rowd.tile([1, TQ], f16, tag="rd")
                        nc.sync.dma_start(out=rd, in_=crh[:])
                        bap = _b.AP(
                            tensor=rd.tensor, offset=rd.offset,
                            ap=[[0, P], [1, TQ]],
                        )
                        cbt = cbtp.tile([P, TQ], f16, tag="cb")
                        nc.sync.dma_start(out=cbt[:], in_=bap)
                        nc.vector.tensor_mul(
                            out=attnT[:, h, jsl], in0=o_ps, in1=cbt[:]
                        )
                        # fill PE slack with out-projection of finished quarters
                        if p3_queue:
                            p3_group(*p3_queue.pop(0))
                    for i in range(4 * J, 4 * J + 4):
                        for n in range(2):
                            p3_queue.append((i, n))
                # drain remaining out-projection groups (quarter 3)
                while p3_queue:
                    p3_group(*p3_queue.pop(0))

    _split_multi_waits(nc, mybir)
    return nc


def _core_inputs(inputs, core):
    """Per-core input map: host-side sharding, fp16 casts, gamma folding."""
    x = np.asarray(inputs["x"], dtype=np.float32)
    gamma = np.asarray(inputs["gamma"], dtype=np.float32)
    beta = np.asarray(inputs["beta"], dtype=np.float32)
    w_qkv = np.asarray(inputs["w_qkv"], dtype=np.float32)
    w_out = np.asarray(inputs["w_out"], dtype=np.float32)

    b, j = core // 2, core % 2
    sl = slice(512 * j, 512 * j + 512)

    def wslice(base):
        wsub = w_qkv[base : base + D][sl]  # [512 out, 1024 in]
        wg = wsub * gamma[None, :]
        bias = wsub @ beta  # [512]
        return np.ascontiguousarray(wg.T).astype(np.float16), bias.astype(np.float32)

    wq16, bq = wslice(0)
    wk16, bk = wslice(D)
    wv16, bvr = wslice(2 * D)
    wl16, bl = wslice(3 * D)
    wp16, bp = wslice(4 * D)

    bqk = np.stack(
        [bq[128 * t : 128 * t + 128] for t in range(4)]
        + [bk[128 * t : 128 * t + 128] for t in range(4)],
        axis=1,
    )
    blp = np.stack(
        [bl[128 * t : 128 * t + 128] for t in range(4)]
        + [bp[128 * t : 128 * t + 128] for t in range(4)],
        axis=1,
    )

    cols = np.r_[512 * j : 512 * j + 512, D + 512 * j : D + 512 * j + 512]
    wo_sel = w_out[:, cols].copy()
    wo_sel[:, 512:] *= 4.0  # attnT stores o'/4; fold the 4x back here
    wo16 = np.ascontiguousarray(wo_sel.T).astype(np.float16)

    kk = np.arange(P)[:, None]
    cc = np.arange(896)[None, :]
    mask16 = (cc >= kk + 384).astype(np.float16)

    return {
        "xT16": np.ascontiguousarray(x[b].T).astype(np.float16),
        "wq16": wq16,
        "wk16": wk16,
        "wv16": wv16,
        "wl16": wl16,
        "wp16": wp16,
        "wo16": wo16,
        "bqk": np.ascontiguousarray(bqk),
        "blp": np.ascontiguousarray(blp),
        "bv16": bvr.astype(np.float16)[None, :],
        "mask16": mask16,
    }


def _run(inputs, trace=False, trace_kwargs=None):
    from concourse.bass_utils import run_bass_kernel_spmd

    beta_zero = not np.any(np.asarray(inputs["beta"]))
    key = ("nc", beta_zero)
    if key not in _CACHED:
        _CACHED[key] = _build_nc(beta_zero)
    nc = _CACHED[key]
    in_maps = [_core_inputs(inputs, c) for c in range(8)]
    res = run_bass_kernel_spmd(
        nc, in_maps, core_ids=list(range(8)), trace=trace,
        **(trace_kwargs or {}),
    )
    x = np.asarray(inputs["x"], dtype=np.float32)
    out = np.empty((B, L, D), dtype=np.float32)
    for b in range(B):
        out[b] = x[b] + (
            res.results[2 * b]["out16"].astype(np.float32)
            + res.results[2 * b + 1]["out16"].astype(np.float32)
        )
    return out, res


def kernel(**inputs) -> np.ndarray:
    out, _ = _run(inputs, trace=False)
    return out
